# revision 46
# baseline (speedup 1.0000x reference)
"""Trainium2 Bass kernel for the CHKGAT ranking problem (raw Bass, no Tile).

Computation (see reference):
    user_embed = entity_embed[users]              # [B, D]
    item_embed = entity_embed[items]              # [B, D]
    buy        = relation_embed[15]               # [D]
    all_items  = entity_embed[:NUM_ITEM]          # [I, D]
    predict        = sigmoid(sum|u+buy-i| + sum(u*i))               # [B]
    ranking_predict= sigmoid(L1(u+buy, all_items) + u@all_items.T)  # [B, I]

Sharding: the item dimension is split across 8 cores (2048 items each). Each
core receives the full entity table (for the on-device index gathers) plus
its contiguous item block; the host concatenates the per-core ranking
slices.

Algorithm (per core). The L1 distance uses the identity
    sum_d |x_d| = 2*sum_d relu(x_d) - sum_d x_d ,
so the per-user elementwise work is ONE fused VectorE/ScalarE relu op
(subtract + max-with-0), and the linear correction terms come cheaply:
  - psum[b, i]  = scores (u.item matmul)                 [start]
                + (-1)-weights matmul over item_T        (= -S_item[i])
                + 2.0-one-hot matmuls over relu streams  (= 2*R[b,i])
  - sigmoid bias adds S_w[b] = sum_d w[d, b] per partition (free in ACT).
Per user b the relu stream rp_b = relu(item_T - w_b) is produced by either
VectorE (fused tensor_scalar, bf16) or ScalarE (Relu activation with bias
-w), split by pairs to balance the two engines; TensorE reduces each stream
with a sliding one-hot (value 2.0) weight so user b's contribution lands in
PSUM row b only. Users b and b+32 run in different PE column-groups
(tile_position) so their matmuls stream concurrently.

This walrus build rejects any compute instruction carrying more than one
semaphore wait, and Tile cannot guarantee that, so the kernel is raw Bass:
every cross-engine dependency is a standalone wait_ge instruction.
"""

from contextlib import ExitStack

import numpy as np

import concourse.bass as bass
import concourse.mybir as mybir
from concourse.bass import IndirectOffsetOnAxis
from concourse.bass_utils import run_bass_kernel_spmd
from concourse.masks import make_identity

NUM_ENTITY = 100000
NUM_RELATION = 16
NUM_ITEM = 16384
DIM = 128
BATCH = 64
N_CORES = 8
IC = NUM_ITEM // N_CORES          # items per core = 2048
NCHUNK = 4                        # psum output chunks per core
CHUNK = IC // NCHUNK              # 512 (one PSUM bank of f32)
NTILE = IC // 128                 # 16 item tiles
NAD = 16                          # relu-stream buffers (manual rotation)
NPAIR = BATCH // 2                # 32 user pairs (b, b+32)
ACT_EVERY = 4                     # every ACT_EVERY-th pair produced on ScalarE
COL_TILING = True                 # run user pairs in two PE column groups

F32 = mybir.dt.float32
BF16 = mybir.dt.bfloat16
I32 = mybir.dt.int32

TRACE = False
LAST_RESULTS = None

_PROG = None


def _plan():
    """Static schedule: pair -> producer, per-user producer ticks, slots."""
    order = []                     # processing order of users: pair p -> (p, p+32)
    for p in range(NPAIR):
        order += [p, p + NPAIR]
    producer = {}                  # user -> "dve" | "act"
    act_pairs = {1, 5, 9, 13, 17, 21, 25}   # 7 pairs on ScalarE, none at tail
    for p in range(NPAIR):
        eng = "act" if p in act_pairs else "dve"
        producer[p] = producer[p + NPAIR] = eng
    # producer streams emit in `order`-restricted-to-their-users sequence
    dve_seq = [u for u in order if producer[u] == "dve"]
    act_seq = [u for u in order if producer[u] == "act"]
    return order, producer, dve_seq, act_seq


def _build_program(repeat=1, mode="full"):
    """repeat>1 re-runs the accumulation phase (timing instrumentation):
    each repeat restarts the PSUM banks, so outputs equal the repeat=1
    result while PE/DVE/ACT steady-state work scales by `repeat`.
    mode: "full" | "pe_only" (producers run once, PE repeats) |
          "prod_only" (producers repeat, PE replaces matmuls with nops)."""
    nc = bass.Bass("TRN2", debug=False)
    AF = mybir.ActivationFunctionType
    OP = mybir.AluOpType

    ent = nc.dram_tensor("entity_embed", [NUM_ENTITY, DIM], F32, kind="ExternalInput").ap()
    iblk = nc.dram_tensor("item_block", [IC, DIM], F32, kind="ExternalInput").ap()
    usr = nc.dram_tensor("users", [BATCH], I32, kind="ExternalInput").ap()
    itm = nc.dram_tensor("items", [BATCH], I32, kind="ExternalInput").ap()
    rel = nc.dram_tensor("relation_embed", [NUM_RELATION, DIM], F32, kind="ExternalInput").ap()
    pred_o = nc.dram_tensor("predict_out", [1, BATCH], F32, kind="ExternalOutput").ap()
    rank_o = nc.dram_tensor("rank_out", [BATCH, IC], F32, kind="ExternalOutput").ap()

    order, producer, dve_seq, act_seq = _plan()
    oidx = {u: i for i, u in enumerate(order)}
    slot = {u: oidx[u] % NAD for u in order}

    # ---- semaphore tick schedule ----
    # SP HWDGE queue (s_hw): even staging tiles + rank chunks 2,3
    # ACT HWDGE queue (s_hw2): idx, rel, odd staging tiles, pred + rank 0,1
    T_HW2_IDX = 32
    T_HW2_REL = 48
    SW_TILES = (11, 13, 15)        # staged via SWDGE after the gathers
    ACT_TILES = tuple(t for t in range(1, NTILE, 2) if t not in SW_TILES)
    T_HW_ALL = 16 * (NTILE // 2) + 32                    # 160
    T_HW2_ALL = 48 + 16 * len(ACT_TILES) + 48            # 176
    T_SW_USER, T_SW_ITEM = 16, 32
    T_SW_ST = lambda t: 32 + 16 * (SW_TILES.index(t) + 1)
    T_PL_IDENT, T_PL_ALL = 1, 3
    # DVE stream: 4 relT + 8 uT + 8 iT, w_T, negw, u_Tb, pd(sub,abs,mult,add),
    # 4 item_T copies, swb copy, then relu ops
    T_DVE_WT = 21
    T_DVE_NEGW = 22
    T_DVE_UTB = 23
    T_DVE_UM1 = 24                         # (u_Tb - 1) scores/-S_item weights
    T_DVE_PD = 28
    T_DVE_COPY = lambda g: 29 + g          # 29..32
    T_DVE_SWB = 33
    dve_idx = {u: i for i, u in enumerate(dve_seq)}
    act_idx = {u: i for i, u in enumerate(act_seq)}
    dve_ad_tick = lambda r, u: T_DVE_SWB + 1 + r * len(dve_seq) + dve_idx[u]
    act_ad_tick = lambda r, u: 2 + r * len(act_seq) + act_idx[u]
    # PE stream: 4 transpose groups, pred mm, S_w mm, then pairs
    T_PE_GRP = lambda g: g + 1
    T_PE_PRED = 5
    T_PE_SW = 6
    T_PE_PAIR = lambda r, p: 7 + r * NPAIR + p
    T_PE_LAST = T_PE_PAIR(repeat - 1, NPAIR - 1)
    if mode == "pe_only":
        prod_repeat, pe_repeat = 1, repeat
    elif mode == "prod_only":
        prod_repeat, pe_repeat = repeat, repeat
    else:
        prod_repeat = pe_repeat = repeat

    # ACT stream: pred sigmoid, relu ops, 4 sigmoids
    T_ACT_PRED = 1
    T_ACT_SIG = lambda c: 2 + prod_repeat * len(act_seq) + c

    def prod_tick(r, u):
        if mode == "pe_only":
            r = 0
        return dve_ad_tick(r, u) if producer[u] == "dve" else act_ad_tick(r, u)

    def war_tick(r, u):
        """Pair tick whose completion frees this user's slot (or None)."""
        g = r * BATCH + oidx[u]
        if g < NAD:
            return None
        gp = (g - NAD) // 2
        return T_PE_PAIR(gp // NPAIR, gp % NPAIR)

    with ExitStack() as ctx:
        sem = lambda n: ctx.enter_context(nc.semaphore(n))
        sb = lambda n, shape, dt: ctx.enter_context(nc.sbuf_tensor(n, shape, dt))
        ps = lambda n, shape: ctx.enter_context(nc.psum_tensor(n, shape, F32))

        s_hw = sem("s_hw")
        s_hw2 = sem("s_hw2")
        s_sw = sem("s_sw")
        s_pool = sem("s_pool")
        s_pe = sem("s_pe")
        s_dve = sem("s_dve")
        s_act = sem("s_act")

        identity = sb("identity", [128, 128], F32)
        # Z32: [D, 63] bf16, zeros except column 31 = 2.0; lhsT slice
        # Z32[:, 31-j : 63-j] puts the 2.0-column at local index j (j=b%32).
        Z32 = sb("Z32", [DIM, 2 * NPAIR - 1], BF16)
        ones_f = sb("ones_f", [DIM, 1], F32)
        users_idx = sb("users_idx", [BATCH, 1], I32)
        items_idx = sb("items_idx", [BATCH, 1], I32)
        rel_sb = sb("rel_sb", [32, DIM], F32)
        st = [sb(f"st{t}", [128, DIM], F32) for t in range(NTILE)]
        user_emb = sb("user_emb", [BATCH, DIM], F32)
        item_emb = sb("item_emb", [BATCH, DIM], F32)
        relT = sb("relT", [DIM, 32], F32)
        u_T = sb("u_T", [DIM, BATCH], F32)
        i_T = sb("i_T", [DIM, BATCH], F32)
        w_T = sb("w_T", [DIM, BATCH], F32)
        negw = sb("negw", [DIM, BATCH], F32)
        u_Tb = sb("u_Tb", [DIM, BATCH], BF16)
        uM1 = sb("uM1", [DIM, BATCH], BF16)
        pd = sb("pd", [DIM, BATCH], F32)
        prod = sb("prod", [DIM, BATCH], F32)
        item_T = sb("item_T", [DIM, IC], BF16)
        ads = [sb(f"ad{i}", [DIM, IC], BF16) for i in range(NAD)]
        swb = sb("swb", [BATCH, 1], F32)
        pred_sb = sb("pred_sb", [1, BATCH], F32)
        ob = [sb(f"ob{c}", [BATCH, CHUNK], F32) for c in range(NCHUNK)]

        tp = [ps(f"tp{g}", [128, 4 * 128]) for g in range(NTILE // 4)]
        pr = [ps(f"pr{c}", [BATCH, CHUNK]) for c in range(NCHUNK)]

        HALF = NPAIR  # 32

        with nc.Block() as block:

            @block.sync
            def _(sync):
                for t in range(0, NTILE, 2):             # even tiles on SP queue
                    sync.dma_start(
                        out=st[t][:, :], in_=iblk[t * 128 : (t + 1) * 128, :]
                    ).then_inc(s_hw, 16)
                for c in (2, 3):
                    sync.wait_ge(s_act, T_ACT_SIG(c))
                    sync.dma_start(
                        out=rank_o[:, c * CHUNK : (c + 1) * CHUNK], in_=ob[c][:, :]
                    ).then_inc(s_hw, 16)
                sync.wait_ge(s_hw, T_HW_ALL)

            @block.gpsimd
            def _(gpsimd):
                gpsimd.memset(identity[:, :], 0.0)
                gpsimd.affine_select(
                    out=identity[:, :],
                    in_=identity[:, :],
                    compare_op=OP.not_equal,
                    fill=1.0,
                    base=0,
                    pattern=[[-1, 128]],
                    channel_multiplier=1,
                ).then_inc(s_pool)                      # 1: identity
                gpsimd.memset(Z32[:, :], 0.0)
                gpsimd.memset(Z32[:, HALF - 1 : HALF], 2.0).then_inc(s_pool)  # 2
                gpsimd.memset(ones_f[:, :], 1.0).then_inc(s_pool)             # 3
                gpsimd.wait_ge(s_hw2, T_HW2_IDX)
                gpsimd.indirect_dma_start(
                    out=user_emb[:, :], out_offset=None, in_=ent,
                    in_offset=IndirectOffsetOnAxis(ap=users_idx[:, :1], axis=0),
                ).then_inc(s_sw, 16)
                gpsimd.indirect_dma_start(
                    out=item_emb[:, :], out_offset=None, in_=ent,
                    in_offset=IndirectOffsetOnAxis(ap=items_idx[:, :1], axis=0),
                ).then_inc(s_sw, 16)
                for t in SW_TILES:                       # tail tiles on SWDGE
                    gpsimd.dma_start(
                        out=st[t][:, :], in_=iblk[t * 128 : (t + 1) * 128, :]
                    ).then_inc(s_sw, 16)

            @block.tensor
            def _(tensor):
                tensor.wait_ge(s_pool, T_PL_IDENT)
                for t in range(NTILE):
                    if t % 2 == 0:
                        tensor.wait_ge(s_hw, 16 * (t // 2 + 1))
                    elif t in SW_TILES:
                        tensor.wait_ge(s_sw, T_SW_ST(t))
                    else:
                        tensor.wait_ge(s_hw2, T_HW2_REL + 16 * ((t + 1) // 2))
                    mm = nc.tensor.transpose(
                        tp[t // 4][:, (t % 4) * 128 : (t % 4 + 1) * 128],
                        st[t][:, :],
                        identity[:, :],
                    )
                    if t % 4 == 3:
                        mm.then_inc(s_pe)               # 1..4
                # predict-branch reduction (tp0 corner; WAR via copy-g0 tick)
                tensor.wait_ge(s_pool, T_PL_ALL)
                tensor.wait_ge(s_dve, T_DVE_COPY(0))
                nc.tensor.matmul(
                    tp[0][:1, :BATCH], ones_f[:, :1], pd[:, :], start=True, stop=True
                ).then_inc(s_pe)                        # 5
                # S_w column (tp1 corner; WAR via copy-g1 tick)
                tensor.wait_ge(s_dve, T_DVE_COPY(1))
                nc.tensor.matmul(
                    tp[1][:BATCH, :1], w_T[:, :], ones_f[:, :1], start=True, stop=True
                ).then_inc(s_pe)                        # 6
                # scores + (-S_item) into the accumulation banks
                tensor.wait_ge(s_dve, T_DVE_COPY(3))  # covers uM1 too
                for r in range(pe_repeat):
                    if mode != "prod_only" or r == 0:
                        for c in range(NCHUNK):
                            nc.tensor.matmul(   # (u-1) weights: scores - S_item
                                pr[c][:, :],
                                uM1[:, :],
                                item_T[:, c * CHUNK : (c + 1) * CHUNK],
                                start=True,
                                stop=False,
                            )
                    # user pair accumulation, two concurrent column groups
                    for p in range(NPAIR):
                        u0, u1 = p, p + HALF
                        for u in (u0, u1):
                            tensor.wait_ge(
                                s_dve if producer[u] == "dve" else s_act,
                                prod_tick(r, u),
                            )
                        if mode == "prod_only":
                            tensor.nop().then_inc(s_pe)
                            continue
                        lhs = Z32[:, HALF - 1 - p : 2 * HALF - 1 - p]
                        stop = p == NPAIR - 1
                        last_mm = None
                        for c in range(NCHUNK):
                            for gi, u in enumerate((u0, u1)):
                                out_ap = pr[c][gi * HALF : (gi + 1) * HALF, :]
                                tpos = (0, gi * HALF)
                                last_mm = nc.tensor.matmul(
                                    out_ap,
                                    lhs,
                                    ads[slot[u]][:, c * CHUNK : (c + 1) * CHUNK],
                                    start=False,
                                    stop=stop,
                                    tile_position=tpos,
                                )
                        last_mm.then_inc(s_pe)          # T_PE_PAIR(r, p)

            @block.vector
            def _(vector):
                vector.wait_ge(s_hw2, T_HW2_REL)
                for dj in range(DIM // 32):
                    nc.vector.transpose(
                        out=relT[dj * 32 : (dj + 1) * 32, 0:32],
                        in_=rel_sb[0:32, dj * 32 : (dj + 1) * 32],
                    ).then_inc(s_dve)                   # 1..4
                vector.wait_ge(s_sw, T_SW_USER)
                for bi in range(BATCH // 32):
                    for dj in range(DIM // 32):
                        nc.vector.transpose(
                            out=u_T[dj * 32 : (dj + 1) * 32, bi * 32 : (bi + 1) * 32],
                            in_=user_emb[bi * 32 : (bi + 1) * 32, dj * 32 : (dj + 1) * 32],
                        ).then_inc(s_dve)               # 5..12
                vector.wait_ge(s_sw, T_SW_ITEM)
                for bi in range(BATCH // 32):
                    for dj in range(DIM // 32):
                        nc.vector.transpose(
                            out=i_T[dj * 32 : (dj + 1) * 32, bi * 32 : (bi + 1) * 32],
                            in_=item_emb[bi * 32 : (bi + 1) * 32, dj * 32 : (dj + 1) * 32],
                        ).then_inc(s_dve)               # 13..20
                nc.vector.tensor_scalar(
                    w_T[:, :], u_T[:, :], relT[:, 15:16], None, OP.add
                ).then_inc(s_dve)                       # 21
                nc.vector.tensor_scalar(
                    negw[:, :], w_T[:, :], -1.0, None, OP.mult
                ).then_inc(s_dve)                       # 22
                nc.vector.tensor_copy(out=u_Tb[:, :], in_=u_T[:, :]).then_inc(s_dve)  # 23
                nc.vector.tensor_scalar(
                    uM1[:, :], u_T[:, :], -1.0, None, OP.add
                ).then_inc(s_dve)                       # 24
                nc.vector.tensor_tensor(
                    out=pd[:, :], in0=w_T[:, :], in1=i_T[:, :], op=OP.subtract
                ).then_inc(s_dve)                       # 24
                nc.vector.tensor_scalar(
                    pd[:, :].bitcast(mybir.dt.uint32),
                    pd[:, :].bitcast(mybir.dt.uint32),
                    0x7FFFFFFF, None, OP.bitwise_and,
                ).then_inc(s_dve)                       # 25
                nc.vector.tensor_tensor(
                    out=prod[:, :], in0=u_T[:, :], in1=i_T[:, :], op=OP.mult
                ).then_inc(s_dve)                       # 26
                nc.vector.tensor_tensor(
                    out=pd[:, :], in0=pd[:, :], in1=prod[:, :], op=OP.add
                ).then_inc(s_dve)                       # 27
                for g in range(NTILE // 4):
                    vector.wait_ge(s_pe, T_PE_GRP(g))
                    nc.vector.tensor_copy(
                        out=item_T[:, g * 512 : (g + 1) * 512], in_=tp[g][:, :]
                    ).then_inc(s_dve)                   # 28..31
                vector.wait_ge(s_pe, T_PE_SW)
                nc.vector.tensor_copy(
                    out=swb[:, :], in_=tp[1][:BATCH, :1]
                ).then_inc(s_dve)                       # 32
                for r in range(prod_repeat):
                    for u in dve_seq:
                        wt = war_tick(r, u)
                        if wt is not None:
                            vector.wait_ge(s_pe, wt)
                        nc.vector.tensor_scalar(       # relu(item_T - w_u)
                            ads[slot[u]][:, :],
                            item_T[:, :],
                            w_T[:, u : u + 1],
                            0.0,
                            OP.subtract,
                            OP.max,
                        ).then_inc(s_dve)               # dve_ad_tick(r, u)

            @block.scalar
            def _(scalar):
                # ACT sequencer doubles as the second HWDGE queue: indices,
                # relation table, odd staging tiles, then (at stream end)
                # the predict + first two ranking outputs.
                scalar.dma_start(out=users_idx[:, :], in_=usr[:, None]).then_inc(s_hw2, 16)
                scalar.dma_start(out=items_idx[:, :], in_=itm[:, None]).then_inc(s_hw2, 16)
                scalar.dma_start(out=rel_sb[:NUM_RELATION, :], in_=rel).then_inc(s_hw2, 16)
                for t in ACT_TILES:                      # odd tiles on ACT queue
                    scalar.dma_start(
                        out=st[t][:, :], in_=iblk[t * 128 : (t + 1) * 128, :]
                    ).then_inc(s_hw2, 16)
                scalar.wait_ge(s_pe, T_PE_PRED)
                nc.scalar.activation(
                    pred_sb[:, :], tp[0][:1, :BATCH], AF.Sigmoid
                ).then_inc(s_act)                       # 1
                scalar.wait_ge(s_dve, T_DVE_COPY(3))    # item_T + negw ready
                for r in range(prod_repeat):
                    for u in act_seq:
                        wt = war_tick(r, u)
                        if wt is not None:
                            scalar.wait_ge(s_pe, wt)
                        nc.scalar.activation(          # relu(item_T - w_u)
                            ads[slot[u]][:, :],
                            item_T[:, :],
                            AF.Relu,
                            bias=negw[:, u : u + 1],
                            scale=1.0,
                        ).then_inc(s_act)               # act_ad_tick(r, u)
                scalar.wait_ge(s_pe, T_PE_LAST)
                scalar.wait_ge(s_dve, T_DVE_SWB)
                for c in range(NCHUNK):
                    nc.scalar.activation(              # sigmoid(x + S_w[b])
                        ob[c][:, :], pr[c][:, :], AF.Sigmoid,
                        bias=swb[:, :1], scale=1.0,
                    ).then_inc(s_act)                   # T_ACT_SIG(c)
                scalar.dma_start(out=pred_o, in_=pred_sb[:, :]).then_inc(s_hw2, 16)
                for c in (0, 1):
                    scalar.dma_start(
                        out=rank_o[:, c * CHUNK : (c + 1) * CHUNK], in_=ob[c][:, :]
                    ).then_inc(s_hw2, 16)
                scalar.wait_ge(s_hw2, T_HW2_ALL)

    return nc


def kernel(users, items, entity_embed, relation_embed):
    global _PROG, LAST_RESULTS
    if _PROG is None:
        _PROG = _build_program()
    nc = _PROG

    users32 = np.ascontiguousarray(np.asarray(users).astype(np.int32))
    items32 = np.ascontiguousarray(np.asarray(items).astype(np.int32))
    ent_np = np.ascontiguousarray(np.asarray(entity_embed, dtype=np.float32))
    rel_np = np.ascontiguousarray(np.asarray(relation_embed, dtype=np.float32))

    in_maps = []
    for c in range(N_CORES):
        in_maps.append(
            {
                "entity_embed": ent_np,
                "item_block": np.ascontiguousarray(ent_np[c * IC : (c + 1) * IC]),
                "users": users32,
                "items": items32,
                "relation_embed": rel_np,
            }
        )

    res = run_bass_kernel_spmd(
        nc, in_maps, core_ids=list(range(N_CORES)), trace=TRACE
    )
    LAST_RESULTS = res

    predict = np.asarray(res.results[0]["predict_out"], dtype=np.float32).reshape(BATCH)
    ranking = np.concatenate(
        [np.asarray(res.results[c]["rank_out"], dtype=np.float32) for c in range(N_CORES)],
        axis=1,
    )
    return predict, ranking
